# revision 19
# baseline (speedup 1.0000x reference)
"""Trainium2 Bass kernel for an EGNN-style GCL layer (gnn_message_passing).

Math (matches the reference):
    edge_in  = concat(x[row], x[col])            # [E, 2D]
    h        = relu(edge_in @ ew1 + eb1)
    edge_feat= relu(h @ ew2 + eb2) * edge_mask   # [E, D]   (output 2)
    agg      = segment_sum(edge_feat, row, N)    # [N, D]
    out      = relu(concat(x, agg) @ nw1 + nb1) @ nw2 + nb2 + x   (output 1)

Strategy (8 NeuronCores, SPMD single NEFF):
  * Host sorts edges by destination (row). Node space padded to 50176 =
    8 * 6272; core c owns nodes [6272c, 6272(c+1)) = 49 windows of 128
    nodes, and all edges whose row lands there. Aggregation therefore
    never crosses cores -> no collectives.
  * Edges of a window are padded to a uniform number of 128-edge tiles
    (T = T_A + T_B, global max over cores/windows) so all cores run the
    identical instruction stream (SPMD requirement); within a window,
    edges with col < 25088 ("A", gathered from the lo half-table) come
    first, then col >= 25088 ("B", hi half-table).  The half-table split
    keeps dma_gather indices inside int16 range.
  * x[col] rows arrive via dma_gather(transpose=True) directly in
    [D, e] layout (fp16).  x[row] likewise from the core-local padded
    node table (indices < 6400).
  * Edge MLP runs in feature-major layout: psum[d, e] accumulating
    lhsT=weight matmuls.  Per 128-edge tile a one-hot P[e, n] (built on
    DVE with is_equal against an iota constant) turns segment-sum into a
    PE matmul: agg[d, nwindow] += feat[e, d].T-style accumulation.
  * Node MLP consumes xT (DMA-transpose) and aggT directly; residual is
    added in fp32 after a PE transpose back to node-major layout.

Everything numeric on-device is fp16 inputs with fp32 PSUM accumulation;
one-hots / iota are exact in fp16.  Host only sorts/pads integer indices
and re-permutes outputs.
"""

import numpy as np

# ---------------------------------------------------------------- constants
N, E, D = 50000, 600000, 128
NCORES = 8
WIN = 128                      # nodes per aggregation window
W = 49                         # windows per core
NPC = WIN * W                  # 6272 nodes per core
NPAD = NPC * NCORES            # 50176
HALF = 25088                   # col-range split (fits int16 with slack)
LO_ROWS = HALF + 128           # lo table rows (zero row at HALF)
HI_ROWS = NPAD - HALF + 128    # hi table rows (25216), tail rows are zero
Z_COL = HALF                   # zero-row index in both half tables
LOC_ROWS = NPC + 128           # per-core row-gather table (zero row at NPC)
Z_ROW = NPC
CHUNK_W = 2                    # windows per gather chunk
F_EDGE = True
F_NODE = True
F_XT = True
F_CHUNKS = 10**9
F_ECOMP = True
NODE_CHUNK = 512


def _ceil_div(a, b):
    return -(-a // b)


def _wrap_idx(stream):
    """int16 stream -> [128, len/16] wrapped layout (16-lane wrap, 8x replicated)."""
    L = stream.shape[0]
    assert L % 16 == 0
    arr = stream.reshape(L // 16, 16).T          # [16, L/16]
    return np.tile(arr, (8, 1)).astype(np.int16)  # [128, L/16]


def _preprocess(edge_index, edge_mask):
    """Sort/pad edges into the uniform per-core schedule.

    Returns (T_A, T_B, per_core list of dicts)."""
    row = edge_index[0].astype(np.int64)
    col = edge_index[1].astype(np.int64)
    mask = edge_mask.reshape(-1).astype(np.float32)

    order = np.argsort(row, kind="stable")
    rs = row[order]
    cores = []
    nA = np.zeros((NCORES, W), np.int64)
    nB = np.zeros((NCORES, W), np.int64)
    for c in range(NCORES):
        lo = np.searchsorted(rs, c * NPC, "left")
        hi = np.searchsorted(rs, (c + 1) * NPC, "left")
        eid = order[lo:hi]
        r = row[eid] - c * NPC
        co = col[eid]
        w = r // WIN
        is_hi = (co >= HALF).astype(np.int64)
        o2 = np.lexsort((is_hi, w))
        eid, r, co, w, is_hi = eid[o2], r[o2], co[o2], w[o2], is_hi[o2]
        for ww in range(W):
            sel = w == ww
            nA[c, ww] = int(np.sum(sel & (is_hi == 0)))
            nB[c, ww] = int(np.sum(sel & (is_hi == 1)))
        cores.append((eid, r, co, w, is_hi))

    T_A = max(1, int(_ceil_div(nA.max(), 128)))
    T_B = max(1, int(_ceil_div(nB.max(), 128)))
    T = T_A + T_B
    S = W * T * 128

    per_core = []
    for c in range(NCORES):
        eid, r, co, w, is_hi = cores[c]
        colA = np.full(W * T_A * 128, Z_COL, np.int64)
        colB = np.full(W * T_B * 128, Z_COL, np.int64)
        rowloc = np.full(S, Z_ROW, np.int64)
        rowrel = np.full(S, -1.0, np.float32)
        maskv = np.zeros(S, np.float32)
        orig = np.full(S, -1, np.int64)
        for ww in range(W):
            sel = w == ww
            a_sel = sel & (is_hi == 0)
            b_sel = sel & (is_hi == 1)
            ka, kb = int(a_sel.sum()), int(b_sel.sum())
            base = ww * T * 128
            # A slots occupy tiles [0, T_A), B tiles [T_A, T)
            colA[ww * T_A * 128: ww * T_A * 128 + ka] = co[a_sel]
            colB[ww * T_B * 128: ww * T_B * 128 + kb] = co[b_sel] - HALF
            rowloc[base: base + ka] = r[a_sel]
            rowloc[base + T_A * 128: base + T_A * 128 + kb] = r[b_sel]
            rowrel[base: base + ka] = (r[a_sel] - ww * WIN).astype(np.float32)
            rowrel[base + T_A * 128: base + T_A * 128 + kb] = (
                r[b_sel] - ww * WIN).astype(np.float32)
            maskv[base: base + ka] = mask[eid[a_sel]]
            maskv[base + T_A * 128: base + T_A * 128 + kb] = mask[eid[b_sel]]
            orig[base: base + ka] = eid[a_sel]
            orig[base + T_A * 128: base + T_A * 128 + kb] = eid[b_sel]
        per_core.append(dict(
            colA=_wrap_idx(colA.astype(np.int16)),
            colB=_wrap_idx(colB.astype(np.int16)),
            rowloc=_wrap_idx(rowloc.astype(np.int16)),
            rowrel=rowrel.reshape(W * T, 128).T.copy(),   # [128, W*T]
            maskt=maskv.reshape(W * T, 128).T.copy(),     # [128, W*T]
            orig=orig,
        ))
    return T_A, T_B, per_core


def _build_bass(T_A, T_B):
    import concourse.tile as tile
    from concourse import bacc, mybir

    f32 = mybir.dt.float32
    f16 = mybir.dt.float16
    i16 = mybir.dt.int16
    Relu = mybir.ActivationFunctionType.Copy  # placeholder, set below
    Relu = mybir.ActivationFunctionType.Relu
    Copy = mybir.ActivationFunctionType.Copy
    Alu = mybir.AluOpType

    T = T_A + T_B
    S = W * T * 128
    LA = W * T_A * 128
    LB = W * T_B * 128

    nc = bacc.Bacc("TRN2", target_bir_lowering=False, debug=False,
                   num_devices=NCORES)

    # ---- dram I/O
    xlo = nc.dram_tensor("xlo", [LO_ROWS, D], f16, kind="ExternalInput")
    xhi = nc.dram_tensor("xhi", [HI_ROWS, D], f16, kind="ExternalInput")
    xlocp = nc.dram_tensor("xlocp", [LOC_ROWS, D], f16, kind="ExternalInput")
    xloc32 = nc.dram_tensor("xloc32", [NPC, D], f32, kind="ExternalInput")
    cidxA = nc.dram_tensor("cidxA", [128, LA // 16], i16, kind="ExternalInput")
    cidxB = nc.dram_tensor("cidxB", [128, LB // 16], i16, kind="ExternalInput")
    rowrel_d = nc.dram_tensor("rowrel", [128, W * T], f16, kind="ExternalInput")
    mask_d = nc.dram_tensor("maskt", [128, W * T], f32, kind="ExternalInput")
    wnames = ["ew1a", "ew1b", "ew2", "nw1a", "nw1b", "nw2"]
    wts_d = {n: nc.dram_tensor(n, [D, D], f16, kind="ExternalInput") for n in wnames}
    bnames = ["eb1", "eb2", "nb1", "nb2"]
    bs_d = {n: nc.dram_tensor(n, [D, 1], f32, kind="ExternalInput") for n in bnames}
    iota_d = nc.dram_tensor("iota", [128, 128], f16, kind="ExternalInput")
    ident_d = nc.dram_tensor("ident", [128, 128], f16, kind="ExternalInput")
    efeat = nc.dram_tensor("efeat", [S, D], f16, kind="ExternalOutput")
    outp = nc.dram_tensor("outp", [NPC, D], f32, kind="ExternalOutput")

    # window-chunk list, e.g. [4,4,...,1] summing to W
    chunks = []
    w0 = 0
    while w0 < W:
        cw = min(CHUNK_W, W - w0)
        chunks.append((w0, cw))
        w0 += cw

    with tile.TileContext(nc) as tc:
        with (
            tc.tile_pool(name="const", bufs=1) as const,
            tc.tile_pool(name="colag", bufs=3) as colag,
            tc.tile_pool(name="colbg", bufs=3) as colbg,
            tc.tile_pool(name="hwork", bufs=3) as hwork,
            tc.tile_pool(name="fwork", bufs=4) as fwork,
            tc.tile_pool(name="owork", bufs=3) as owork,
            tc.tile_pool(name="p1", bufs=2, space="PSUM") as p1,
            tc.tile_pool(name="p2", bufs=2, space="PSUM") as p2,
            tc.tile_pool(name="pft", bufs=2, space="PSUM") as pft,
            tc.tile_pool(name="pagg", bufs=2, space="PSUM") as pagg,
        ):
            # ---------- constants / tables into SBUF
            wt = {n: const.tile([D, D], f16, tag=n, name=n) for n in wnames}
            for n in wnames:
                nc.sync.dma_start(out=wt[n][:], in_=wts_d[n][:])
            bt = {n: const.tile([D, 1], f32, tag=n, name=n) for n in bnames}
            for n in bnames:
                nc.sync.dma_start(out=bt[n][:], in_=bs_d[n][:])
            iota_t = const.tile([128, 128], f16, tag="iota")
            nc.sync.dma_start(out=iota_t[:], in_=iota_d[:])
            ident_t = const.tile([128, 128], f16, tag="ident")
            nc.sync.dma_start(out=ident_t[:], in_=ident_d[:])
            rowrel_t = const.tile([128, W * T], f16, tag="rowrel")
            nc.sync.dma_start(out=rowrel_t[:], in_=rowrel_d[:])
            mask_t = const.tile([128, W * T], f32, tag="mask")
            nc.sync.dma_start(out=mask_t[:], in_=mask_d[:])
            idxA_t = const.tile([128, LA // 16], i16, tag="idxA")
            nc.sync.dma_start(out=idxA_t[:], in_=cidxA[:])
            idxB_t = const.tile([128, LB // 16], i16, tag="idxB")
            nc.sync.dma_start(out=idxB_t[:], in_=cidxB[:])
            xT = const.tile([128, NPC], f16, tag="xT")
            if F_XT:
                nc.sync.dma_start(out=xT[:], in_=xlocp[:NPC, :], transpose=True)
            else:
                nc.vector.memset(xT[:], 0.0)
            agg32 = const.tile([128, NPC], f32, tag="agg32")
            nc.vector.memset(agg32[:], 0.0)
            aggh = const.tile([128, NPC], f16, tag="aggh")
            # A_alt[n, d] = x_loc @ ew1a, per 128-node window ([n, d] tiles
            # side by side) -- the row-side L1 term, expanded to edges later
            # via the transposed one-hot (saves the on-device row gather).
            a_alt = const.tile([128, NPC], f16, tag="a_alt")
            if F_EDGE:
                for w in range(W):
                    psA = pft.tile([128, 128], f32, tag="pft", name="psA")
                    nc.tensor.matmul(psA[:], xT[:, w * 128:(w + 1) * 128],
                                     wt["ew1a"][:], start=True, stop=True)
                    nc.vector.tensor_copy(out=a_alt[:, w * 128:(w + 1) * 128],
                                          in_=psA[:])

            evac_flip = [0]

            # ---------- edge phase
            for (w0, cw) in (chunks if F_EDGE else [])[:F_CHUNKS]:
                na = cw * T_A * 128
                nb = cw * T_B * 128
                at = colag.tile([128, na], f16, tag="colabuf")
                bt_g = colbg.tile([128, nb], f16, tag="colbbuf")
                nc.gpsimd.dma_gather(
                    out_ap=at[:].rearrange("p (o n) -> p o n", o=1),
                    in_ap=xlo[:],
                    idxs_ap=idxA_t[:, w0 * T_A * 8: (w0 + cw) * T_A * 8],
                    num_idxs=na, num_idxs_reg=na,
                    elem_size=D, transpose=True, single_packet=False)
                nc.gpsimd.dma_gather(
                    out_ap=bt_g[:].rearrange("p (o n) -> p o n", o=1),
                    in_ap=xhi[:],
                    idxs_ap=idxB_t[:, w0 * T_B * 8: (w0 + cw) * T_B * 8],
                    num_idxs=nb, num_idxs_reg=nb,
                    elem_size=D, transpose=True, single_packet=False)

                for wl in (range(cw) if F_ECOMP else []):
                    w = w0 + wl
                    pagg_w = pagg.tile([128, 128], f32, tag="pagg")
                    # groups of up to 4 tiles; never cross the A/B boundary so
                    # each group's gathered cols are one contiguous rhs slab
                    groups = (
                        [list(range(g, min(g + 4, T_A))) for g in range(0, T_A, 4)]
                        + [list(range(g, min(g + 4, T))) for g in range(T_A, T, 4)])
                    for grp in groups:
                        g0 = grp[0]
                        gl = len(grp) * 128
                        if g0 < T_A:
                            o = (wl * T_A + g0) * 128
                            col_slab = at[:, o:o + gl]
                        else:
                            o = (wl * T_B + (g0 - T_A)) * 128
                            col_slab = bt_g[:, o:o + gl]
                        # one-hots for the group + their transposes (batched)
                        Ps = {}
                        psTg = pft.tile([128, gl], f16, tag="pft", name="psTg")
                        for k, j in enumerate(grp):
                            wtile = w * T + j
                            P_j = fwork.tile([128, 128], f16, tag="P", name="P_j")
                            nc.vector.tensor_tensor(
                                out=P_j[:],
                                in0=rowrel_t[:, wtile:wtile + 1].to_broadcast([128, 128]),
                                in1=iota_t[:],
                                op=Alu.is_equal)
                            Ps[j] = P_j
                            nc.tensor.transpose(psTg[:, k * 128:(k + 1) * 128],
                                                P_j[:], ident_t[:])
                        PTg = fwork.tile([128, gl], f16, tag="PT", name="PTg")
                        nc.vector.tensor_copy(out=PTg[:], in_=psTg[:])
                        # L1: one expansion matmul + one col matmul per group
                        ps1 = p1.tile([128, gl], f32, tag="ps1")
                        nc.tensor.matmul(ps1[:], a_alt[:, w * 128:(w + 1) * 128],
                                         PTg[:], start=True, stop=False)
                        nc.tensor.matmul(ps1[:], wt["ew1b"][:], col_slab,
                                         start=False, stop=True)
                        h1 = hwork.tile([128, gl], f16, tag="h1")
                        nc.scalar.activation(h1[:], ps1[:], Relu, bias=bt["eb1"][:, :1])
                        ps2 = p2.tile([128, gl], f32, tag="ps2")
                        nc.tensor.matmul(ps2[:], wt["ew2"][:], h1[:],
                                         start=True, stop=True)
                        h2 = hwork.tile([128, gl], f16, tag="h2")
                        nc.scalar.activation(h2[:], ps2[:], Relu, bias=bt["eb2"][:, :1])
                        # transpose back to [e, d], mask, one store per group
                        pfg = pft.tile([128, gl], f16, tag="pft", name="pfg")
                        fbg = fwork.tile([128, gl], f16, tag="fb", name="fbg")
                        for k, j in enumerate(grp):
                            wtile = w * T + j
                            nc.tensor.transpose(pfg[:, k * 128:(k + 1) * 128],
                                                h2[:, k * 128:(k + 1) * 128],
                                                ident_t[:])
                            if evac_flip[0] % 2 == 0:
                                nc.vector.tensor_scalar(
                                    out=fbg[:, k * 128:(k + 1) * 128],
                                    in0=pfg[:, k * 128:(k + 1) * 128],
                                    scalar1=mask_t[:, wtile:wtile + 1],
                                    scalar2=None, op0=Alu.mult)
                            else:
                                nc.scalar.activation(
                                    fbg[:, k * 128:(k + 1) * 128],
                                    pfg[:, k * 128:(k + 1) * 128], Copy,
                                    scale=mask_t[:, wtile:wtile + 1])
                            evac_flip[0] += 1
                            nc.tensor.matmul(pagg_w[:],
                                             fbg[:, k * 128:(k + 1) * 128],
                                             Ps[j][:],
                                             start=(j == 0), stop=(j == T - 1))
                        s0 = (w * T + g0) * 128
                        nc.sync.dma_start(
                            out=efeat[s0:s0 + gl, :].rearrange(
                                "(g e) d -> e g d", e=128),
                            in_=fbg[:].rearrange("p (g d) -> p g d", g=len(grp)))
                    nc.vector.tensor_tensor(
                        out=agg32[:, w * 128:(w + 1) * 128],
                        in0=pagg_w[:],
                        in1=agg32[:, w * 128:(w + 1) * 128],
                        op=Alu.add)

            # ---------- node phase
            nc.vector.tensor_copy(out=aggh[:], in_=agg32[:])
            for c0 in (range(0, NPC, NODE_CHUNK) if F_NODE else []):
                L = min(NODE_CHUNK, NPC - c0)
                pn1 = p1.tile([128, L], f32, tag="ps1")
                nc.tensor.matmul(pn1[:], wt["nw1a"][:], xT[:, c0:c0 + L],
                                 start=True, stop=False)
                nc.tensor.matmul(pn1[:], wt["nw1b"][:], aggh[:, c0:c0 + L],
                                 start=False, stop=True)
                hn = hwork.tile([128, L], f16, tag="h1")
                nc.scalar.activation(hn[:], pn1[:], Relu, bias=bt["nb1"][:, :1])
                pn2 = p2.tile([128, L], f32, tag="ps2")
                nc.tensor.matmul(pn2[:], wt["nw2"][:], hn[:], start=True, stop=True)
                preo = hwork.tile([128, L], f16, tag="h2")
                nc.vector.tensor_scalar(out=preo[:], in0=pn2[:],
                                        scalar1=bt["nb2"][:, :1], scalar2=None,
                                        op0=Alu.add)
                for j in range(L // 128):
                    n0 = c0 + j * 128
                    pt = pft.tile([128, 128], f16, tag="pft")
                    nc.tensor.transpose(pt[:], preo[:, j * 128:(j + 1) * 128],
                                        ident_t[:])
                    xres = owork.tile([128, 128], f32, tag="xres")
                    nc.sync.dma_start(out=xres[:], in_=xloc32[n0:n0 + 128, :])
                    ot = owork.tile([128, 128], f32, tag="ot")
                    nc.vector.tensor_tensor(out=ot[:], in0=pt[:], in1=xres[:],
                                            op=Alu.add)
                    nc.sync.dma_start(out=outp[n0:n0 + 128, :], in_=ot[:])
    nc.compile()
    return nc


def _make_inputs(x, per_core_sched, ew1, eb1, ew2, eb2, nw1, nb1, nw2, nb2):
    f16 = np.float16
    xpad = np.zeros((NPAD, D), np.float32)
    xpad[:N] = x
    x16 = xpad.astype(f16)
    xlo = np.zeros((LO_ROWS, D), f16)
    xlo[:HALF] = x16[:HALF]
    xhi = np.zeros((HI_ROWS, D), f16)
    xhi[:NPAD - HALF] = x16[HALF:]
    shared = dict(
        xlo=xlo, xhi=xhi,
        iota=np.broadcast_to(np.arange(128, dtype=f16), (128, 128)).copy(),
        ident=np.eye(128, dtype=f16),
        ew1a=ew1[:D].astype(f16), ew1b=ew1[D:].astype(f16),
        ew2=ew2.astype(f16),
        nw1a=nw1[:D].astype(f16), nw1b=nw1[D:].astype(f16),
        nw2=nw2.astype(f16),
        eb1=eb1.reshape(D, 1).astype(np.float32),
        eb2=eb2.reshape(D, 1).astype(np.float32),
        nb1=nb1.reshape(D, 1).astype(np.float32),
        nb2=nb2.reshape(D, 1).astype(np.float32),
    )
    in_maps = []
    for c in range(NCORES):
        sc = per_core_sched[c]
        xlocp = np.zeros((LOC_ROWS, D), f16)
        xlocp[:NPC] = x16[c * NPC:(c + 1) * NPC]
        m = dict(shared)
        m.update(
            xlocp=xlocp,
            xloc32=xpad[c * NPC:(c + 1) * NPC].copy(),
            cidxA=sc["colA"], cidxB=sc["colB"],
            rowrel=sc["rowrel"].astype(f16), maskt=sc["maskt"],
        )
        in_maps.append(m)
    return in_maps


_CACHE = {}
LAST_RESULT = None


def kernel(x, edge_index, edge_mask, ew1, eb1, ew2, eb2, nw1, nb1, nw2, nb2):
    x = np.asarray(x, np.float32)
    edge_index = np.asarray(edge_index)
    edge_mask = np.asarray(edge_mask, np.float32)

    T_A, T_B, sched = _preprocess(edge_index, edge_mask)
    in_maps = _make_inputs(np.asarray(x, np.float32), sched,
                           np.asarray(ew1, np.float32), np.asarray(eb1, np.float32),
                           np.asarray(ew2, np.float32), np.asarray(eb2, np.float32),
                           np.asarray(nw1, np.float32), np.asarray(nb1, np.float32),
                           np.asarray(nw2, np.float32), np.asarray(nb2, np.float32))

    key = (T_A, T_B)
    if key not in _CACHE:
        _CACHE[key] = _build_bass(T_A, T_B)
    nc = _CACHE[key]

    from concourse.bass_utils import run_bass_kernel_spmd
    global LAST_RESULT
    LAST_RESULT = run_bass_kernel_spmd(nc, in_maps, list(range(NCORES)))
    res = LAST_RESULT.results

    out = np.zeros((N, D), np.float32)
    edge_feat = np.zeros((E, D), np.float32)
    for c in range(NCORES):
        n0 = c * NPC
        n1 = min((c + 1) * NPC, N)
        out[n0:n1] = res[c]["outp"][:n1 - n0]
        orig = sched[c]["orig"]
        sel = orig >= 0
        edge_feat[orig[sel]] = res[c]["efeat"][sel].astype(np.float32)
    return out, edge_feat


# revision 21
# speedup vs baseline: 1.2791x; 1.2791x over previous
"""Trainium2 Bass kernel for an EGNN-style GCL layer (gnn_message_passing).

Math (matches the reference):
    edge_in  = concat(x[row], x[col])            # [E, 2D]
    h        = relu(edge_in @ ew1 + eb1)
    edge_feat= relu(h @ ew2 + eb2) * edge_mask   # [E, D]   (output 2)
    agg      = segment_sum(edge_feat, row, N)    # [N, D]
    out      = relu(concat(x, agg) @ nw1 + nb1) @ nw2 + nb2 + x   (output 1)

Strategy (8 NeuronCores, SPMD single NEFF):
  * Host sorts edges by destination (row). Node space padded to 50176 =
    8 * 6272; core c owns nodes [6272c, 6272(c+1)) = 49 windows of 128
    nodes, and all edges whose row lands there. Aggregation therefore
    never crosses cores -> no collectives.
  * Edges of a window are padded to a uniform number of 128-edge tiles
    (T = T_A + T_B, global max over cores/windows) so all cores run the
    identical instruction stream (SPMD requirement); within a window,
    edges with col < 25088 ("A", gathered from the lo half-table) come
    first, then col >= 25088 ("B", hi half-table).  The half-table split
    keeps dma_gather indices inside int16 range.
  * x[col] rows arrive via dma_gather(transpose=True) directly in
    [D, e] layout (fp16).  x[row] likewise from the core-local padded
    node table (indices < 6400).
  * Edge MLP runs in feature-major layout: psum[d, e] accumulating
    lhsT=weight matmuls.  Per 128-edge tile a one-hot P[e, n] (built on
    DVE with is_equal against an iota constant) turns segment-sum into a
    PE matmul: agg[d, nwindow] += feat[e, d].T-style accumulation.
  * Node MLP consumes xT (DMA-transpose) and aggT directly; residual is
    added in fp32 after a PE transpose back to node-major layout.

Everything numeric on-device is fp16 inputs with fp32 PSUM accumulation;
one-hots / iota are exact in fp16.  Host only sorts/pads integer indices
and re-permutes outputs.
"""

import numpy as np

# ---------------------------------------------------------------- constants
N, E, D = 50000, 600000, 128
NCORES = 8
WIN = 128                      # nodes per aggregation window
W = 49                         # windows per core
NPC = WIN * W                  # 6272 nodes per core
NPAD = NPC * NCORES            # 50176
HALF = 25088                   # col-range split (fits int16 with slack)
LO_ROWS = HALF + 128           # lo table rows (zero row at HALF)
HI_ROWS = NPAD - HALF + 128    # hi table rows (25216), tail rows are zero
Z_COL = HALF                   # zero-row index in both half tables
LOC_ROWS = NPC + 128           # per-core row-gather table (zero row at NPC)
Z_ROW = NPC
CHUNK_W = 1                    # windows per gather chunk
F_EDGE = True
F_NODE = True
F_XT = True
F_CHUNKS = 10**9
F_ECOMP = True
NODE_CHUNK = 512


def _ceil_div(a, b):
    return -(-a // b)


def _wrap_idx(stream):
    """int16 stream -> [128, len/16] wrapped layout (16-lane wrap, 8x replicated)."""
    L = stream.shape[0]
    assert L % 16 == 0
    arr = stream.reshape(L // 16, 16).T          # [16, L/16]
    return np.tile(arr, (8, 1)).astype(np.int16)  # [128, L/16]


def _preprocess(edge_index, edge_mask):
    """Sort/pad edges into the uniform per-core schedule.

    Returns (T_A, T_B, per_core list of dicts)."""
    row = edge_index[0].astype(np.int64)
    col = edge_index[1].astype(np.int64)
    mask = edge_mask.reshape(-1).astype(np.float32)

    order = np.argsort(row, kind="stable")
    rs = row[order]
    cores = []
    nA = np.zeros((NCORES, W), np.int64)
    nB = np.zeros((NCORES, W), np.int64)
    for c in range(NCORES):
        lo = np.searchsorted(rs, c * NPC, "left")
        hi = np.searchsorted(rs, (c + 1) * NPC, "left")
        eid = order[lo:hi]
        r = row[eid] - c * NPC
        co = col[eid]
        w = r // WIN
        is_hi = (co >= HALF).astype(np.int64)
        o2 = np.lexsort((is_hi, w))
        eid, r, co, w, is_hi = eid[o2], r[o2], co[o2], w[o2], is_hi[o2]
        for ww in range(W):
            sel = w == ww
            nA[c, ww] = int(np.sum(sel & (is_hi == 0)))
            nB[c, ww] = int(np.sum(sel & (is_hi == 1)))
        cores.append((eid, r, co, w, is_hi))

    T_A = max(1, int(_ceil_div(nA.max(), 128)))
    T_B = max(1, int(_ceil_div(nB.max(), 128)))
    T = T_A + T_B
    S = W * T * 128

    per_core = []
    for c in range(NCORES):
        eid, r, co, w, is_hi = cores[c]
        colA = np.full(W * T_A * 128, Z_COL, np.int64)
        colB = np.full(W * T_B * 128, Z_COL, np.int64)
        rowloc = np.full(S, Z_ROW, np.int64)
        rowrel = np.full(S, -1.0, np.float32)
        maskv = np.zeros(S, np.float32)
        orig = np.full(S, -1, np.int64)
        for ww in range(W):
            sel = w == ww
            a_sel = sel & (is_hi == 0)
            b_sel = sel & (is_hi == 1)
            ka, kb = int(a_sel.sum()), int(b_sel.sum())
            base = ww * T * 128
            # A slots occupy tiles [0, T_A), B tiles [T_A, T)
            colA[ww * T_A * 128: ww * T_A * 128 + ka] = co[a_sel]
            colB[ww * T_B * 128: ww * T_B * 128 + kb] = co[b_sel] - HALF
            rowloc[base: base + ka] = r[a_sel]
            rowloc[base + T_A * 128: base + T_A * 128 + kb] = r[b_sel]
            rowrel[base: base + ka] = (r[a_sel] - ww * WIN).astype(np.float32)
            rowrel[base + T_A * 128: base + T_A * 128 + kb] = (
                r[b_sel] - ww * WIN).astype(np.float32)
            maskv[base: base + ka] = mask[eid[a_sel]]
            maskv[base + T_A * 128: base + T_A * 128 + kb] = mask[eid[b_sel]]
            orig[base: base + ka] = eid[a_sel]
            orig[base + T_A * 128: base + T_A * 128 + kb] = eid[b_sel]
        per_core.append(dict(
            colA=_wrap_idx(colA.astype(np.int16)),
            colB=_wrap_idx(colB.astype(np.int16)),
            rowloc=_wrap_idx(rowloc.astype(np.int16)),
            rowrel=rowrel.reshape(W * T, 128).T.copy(),   # [128, W*T]
            maskt=maskv.reshape(W * T, 128).T.copy(),     # [128, W*T]
            orig=orig,
        ))
    return T_A, T_B, per_core


def _build_bass(T_A, T_B):
    import concourse.tile as tile
    from concourse import bacc, mybir

    f32 = mybir.dt.float32
    f16 = mybir.dt.float16
    i16 = mybir.dt.int16
    Relu = mybir.ActivationFunctionType.Copy  # placeholder, set below
    Relu = mybir.ActivationFunctionType.Relu
    Copy = mybir.ActivationFunctionType.Copy
    Alu = mybir.AluOpType

    T = T_A + T_B
    S = W * T * 128
    LA = W * T_A * 128
    LB = W * T_B * 128

    nc = bacc.Bacc("TRN2", target_bir_lowering=False, debug=False,
                   num_devices=NCORES)

    # ---- dram I/O
    xlo = nc.dram_tensor("xlo", [LO_ROWS, D], f16, kind="ExternalInput")
    xhi = nc.dram_tensor("xhi", [HI_ROWS, D], f16, kind="ExternalInput")
    xlocp = nc.dram_tensor("xlocp", [LOC_ROWS, D], f16, kind="ExternalInput")
    xloc32 = nc.dram_tensor("xloc32", [NPC, D], f32, kind="ExternalInput")
    cidxA = nc.dram_tensor("cidxA", [128, LA // 16], i16, kind="ExternalInput")
    cidxB = nc.dram_tensor("cidxB", [128, LB // 16], i16, kind="ExternalInput")
    rowrel_d = nc.dram_tensor("rowrel", [128, W * T], f16, kind="ExternalInput")
    mask_d = nc.dram_tensor("maskt", [128, W * T], f32, kind="ExternalInput")
    wnames = ["ew1a", "ew1b", "ew2", "nw1a", "nw1b", "nw2"]
    wts_d = {n: nc.dram_tensor(n, [D, D], f16, kind="ExternalInput") for n in wnames}
    bnames = ["eb1", "eb2", "nb1", "nb2"]
    bs_d = {n: nc.dram_tensor(n, [D, 1], f32, kind="ExternalInput") for n in bnames}
    iota_d = nc.dram_tensor("iota", [128, 128], f16, kind="ExternalInput")
    ident_d = nc.dram_tensor("ident", [128, 128], f16, kind="ExternalInput")
    efeat = nc.dram_tensor("efeat", [S, D], f16, kind="ExternalOutput")
    outp = nc.dram_tensor("outp", [NPC, D], f32, kind="ExternalOutput")

    # window-chunk list, e.g. [4,4,...,1] summing to W
    chunks = []
    w0 = 0
    while w0 < W:
        cw = min(CHUNK_W, W - w0)
        chunks.append((w0, cw))
        w0 += cw

    with tile.TileContext(nc) as tc:
        with (
            tc.tile_pool(name="const", bufs=1) as const,
            tc.tile_pool(name="colag", bufs=6) as colag,
            tc.tile_pool(name="colbg", bufs=6) as colbg,
            tc.tile_pool(name="hwork", bufs=3) as hwork,
            tc.tile_pool(name="fwork", bufs=4) as fwork,
            tc.tile_pool(name="owork", bufs=3) as owork,
            tc.tile_pool(name="p1", bufs=2, space="PSUM") as p1,
            tc.tile_pool(name="p2", bufs=2, space="PSUM") as p2,
            tc.tile_pool(name="pft", bufs=3, space="PSUM") as pft,
            tc.tile_pool(name="pagg", bufs=1, space="PSUM") as pagg,
        ):
            # ---------- constants / tables into SBUF
            wt = {n: const.tile([D, D], f16, tag=n, name=n) for n in wnames}
            for n in wnames:
                nc.sync.dma_start(out=wt[n][:], in_=wts_d[n][:])
            bt = {n: const.tile([D, 1], f32, tag=n, name=n) for n in bnames}
            for n in bnames:
                nc.sync.dma_start(out=bt[n][:], in_=bs_d[n][:])
            iota_t = const.tile([128, 128], f16, tag="iota")
            nc.sync.dma_start(out=iota_t[:], in_=iota_d[:])
            ident_t = const.tile([128, 128], f16, tag="ident")
            nc.sync.dma_start(out=ident_t[:], in_=ident_d[:])
            rowrel_t = const.tile([128, W * T], f16, tag="rowrel")
            nc.sync.dma_start(out=rowrel_t[:], in_=rowrel_d[:])
            mask_t = const.tile([128, W * T], f32, tag="mask")
            nc.sync.dma_start(out=mask_t[:], in_=mask_d[:])
            idxA_t = const.tile([128, LA // 16], i16, tag="idxA")
            nc.sync.dma_start(out=idxA_t[:], in_=cidxA[:])
            idxB_t = const.tile([128, LB // 16], i16, tag="idxB")
            nc.sync.dma_start(out=idxB_t[:], in_=cidxB[:])
            xT = const.tile([128, NPC], f16, tag="xT")
            if F_XT:
                nc.sync.dma_start(out=xT[:], in_=xlocp[:NPC, :], transpose=True)
            else:
                nc.vector.memset(xT[:], 0.0)
            agg32 = const.tile([128, NPC], f32, tag="agg32")
            nc.vector.memset(agg32[:], 0.0)
            aggh = const.tile([128, NPC], f16, tag="aggh")
            # A_alt[n, d] = x_loc @ ew1a, per 128-node window ([n, d] tiles
            # side by side) -- the row-side L1 term, expanded to edges later
            # via the transposed one-hot (saves the on-device row gather).
            a_alt = const.tile([128, NPC], f16, tag="a_alt")
            if F_EDGE:
                for w in range(W):
                    psA = pft.tile([128, 128], f32, tag="pft", name="psA")
                    nc.tensor.matmul(psA[:], xT[:, w * 128:(w + 1) * 128],
                                     wt["ew1a"][:], start=True, stop=True)
                    nc.vector.tensor_copy(out=a_alt[:, w * 128:(w + 1) * 128],
                                          in_=psA[:])

            evac_flip = [0]

            # ---------- edge phase
            for (w0, cw) in (chunks if F_EDGE else [])[:F_CHUNKS]:
                na = cw * T_A * 128
                nb = cw * T_B * 128
                at = colag.tile([128, na], f16, tag="colabuf")
                bt_g = colbg.tile([128, nb], f16, tag="colbbuf")
                nc.gpsimd.dma_gather(
                    out_ap=at[:].rearrange("p (o n) -> p o n", o=1),
                    in_ap=xlo[:],
                    idxs_ap=idxA_t[:, w0 * T_A * 8: (w0 + cw) * T_A * 8],
                    num_idxs=na, num_idxs_reg=na,
                    elem_size=D, transpose=True, single_packet=False)
                nc.gpsimd.dma_gather(
                    out_ap=bt_g[:].rearrange("p (o n) -> p o n", o=1),
                    in_ap=xhi[:],
                    idxs_ap=idxB_t[:, w0 * T_B * 8: (w0 + cw) * T_B * 8],
                    num_idxs=nb, num_idxs_reg=nb,
                    elem_size=D, transpose=True, single_packet=False)

                for wl in (range(cw) if F_ECOMP else []):
                    w = w0 + wl
                    pagg_w = pagg.tile([128, 128], f32, tag="pagg")
                    # groups of up to 4 tiles share one [128, <=512] psum
                    groups = [list(range(g, min(g + 4, T))) for g in range(0, T, 4)]
                    for grp in groups:
                        gl = len(grp) * 128
                        ps1 = p1.tile([128, gl], f32, tag="ps1")
                        Ps = {}
                        for k, j in enumerate(grp):
                            wtile = w * T + j
                            if j < T_A:
                                o = (wl * T_A + j) * 128
                                rhs_col = at[:, o:o + 128]
                            else:
                                o = (wl * T_B + (j - T_A)) * 128
                                rhs_col = bt_g[:, o:o + 128]
                            P_j = fwork.tile([128, 128], f16, tag="P", name="P_j")
                            nc.vector.tensor_tensor(
                                out=P_j[:],
                                in0=rowrel_t[:, wtile:wtile + 1].to_broadcast([128, 128]),
                                in1=iota_t[:],
                                op=Alu.is_equal)
                            Ps[j] = P_j
                            psT = pft.tile([128, 128], f16, tag="pft", name="psT")
                            nc.tensor.transpose(psT[:], P_j[:], ident_t[:])
                            PT_j = fwork.tile([128, 128], f16, tag="PT", name="PT_j")
                            nc.vector.tensor_copy(out=PT_j[:], in_=psT[:])
                            nc.tensor.matmul(ps1[:, k * 128:(k + 1) * 128],
                                             a_alt[:, w * 128:(w + 1) * 128], PT_j[:],
                                             start=True, stop=False)
                            nc.tensor.matmul(ps1[:, k * 128:(k + 1) * 128],
                                             wt["ew1b"][:], rhs_col,
                                             start=False, stop=True)
                        h1 = hwork.tile([128, gl], f16, tag="h1")
                        nc.scalar.activation(h1[:], ps1[:], Relu, bias=bt["eb1"][:, :1])
                        ps2 = p2.tile([128, gl], f32, tag="ps2")
                        nc.tensor.matmul(ps2[:], wt["ew2"][:], h1[:],
                                         start=True, stop=True)
                        h2 = hwork.tile([128, gl], f16, tag="h2")
                        nc.scalar.activation(h2[:], ps2[:], Relu, bias=bt["eb2"][:, :1])
                        for k, j in enumerate(grp):
                            wtile = w * T + j
                            P_j = Ps[j]
                            pf = pft.tile([128, 128], f16, tag="pft")
                            nc.tensor.transpose(pf[:], h2[:, k * 128:(k + 1) * 128],
                                                ident_t[:])
                            fb = fwork.tile([128, 128], f16, tag="fb")
                            if evac_flip[0] % 2 == 0:
                                nc.vector.tensor_scalar(
                                    out=fb[:], in0=pf[:],
                                    scalar1=mask_t[:, wtile:wtile + 1],
                                    scalar2=None, op0=Alu.mult)
                            else:
                                nc.scalar.activation(
                                    fb[:], pf[:], Copy,
                                    scale=mask_t[:, wtile:wtile + 1])
                            evac_flip[0] += 1
                            nc.sync.dma_start(
                                out=efeat[wtile * 128:(wtile + 1) * 128, :],
                                in_=fb[:])
                            nc.tensor.matmul(pagg_w[:], fb[:], P_j[:],
                                             start=(j == 0), stop=(j == T - 1))
                    nc.vector.tensor_tensor(
                        out=agg32[:, w * 128:(w + 1) * 128],
                        in0=pagg_w[:],
                        in1=agg32[:, w * 128:(w + 1) * 128],
                        op=Alu.add)

            # ---------- node phase
            nc.vector.tensor_copy(out=aggh[:], in_=agg32[:])
            for c0 in (range(0, NPC, NODE_CHUNK) if F_NODE else []):
                L = min(NODE_CHUNK, NPC - c0)
                pn1 = p1.tile([128, L], f32, tag="ps1")
                nc.tensor.matmul(pn1[:], wt["nw1a"][:], xT[:, c0:c0 + L],
                                 start=True, stop=False)
                nc.tensor.matmul(pn1[:], wt["nw1b"][:], aggh[:, c0:c0 + L],
                                 start=False, stop=True)
                hn = hwork.tile([128, L], f16, tag="h1")
                nc.scalar.activation(hn[:], pn1[:], Relu, bias=bt["nb1"][:, :1])
                pn2 = p2.tile([128, L], f32, tag="ps2")
                nc.tensor.matmul(pn2[:], wt["nw2"][:], hn[:], start=True, stop=True)
                preo = hwork.tile([128, L], f16, tag="h2")
                nc.vector.tensor_scalar(out=preo[:], in0=pn2[:],
                                        scalar1=bt["nb2"][:, :1], scalar2=None,
                                        op0=Alu.add)
                for j in range(L // 128):
                    n0 = c0 + j * 128
                    pt = pft.tile([128, 128], f16, tag="pft")
                    nc.tensor.transpose(pt[:], preo[:, j * 128:(j + 1) * 128],
                                        ident_t[:])
                    xres = owork.tile([128, 128], f32, tag="xres")
                    nc.sync.dma_start(out=xres[:], in_=xloc32[n0:n0 + 128, :])
                    ot = owork.tile([128, 128], f32, tag="ot")
                    nc.vector.tensor_tensor(out=ot[:], in0=pt[:], in1=xres[:],
                                            op=Alu.add)
                    nc.sync.dma_start(out=outp[n0:n0 + 128, :], in_=ot[:])
    nc.compile()
    return nc


def _make_inputs(x, per_core_sched, ew1, eb1, ew2, eb2, nw1, nb1, nw2, nb2):
    f16 = np.float16
    xpad = np.zeros((NPAD, D), np.float32)
    xpad[:N] = x
    x16 = xpad.astype(f16)
    xlo = np.zeros((LO_ROWS, D), f16)
    xlo[:HALF] = x16[:HALF]
    xhi = np.zeros((HI_ROWS, D), f16)
    xhi[:NPAD - HALF] = x16[HALF:]
    shared = dict(
        xlo=xlo, xhi=xhi,
        iota=np.broadcast_to(np.arange(128, dtype=f16), (128, 128)).copy(),
        ident=np.eye(128, dtype=f16),
        ew1a=ew1[:D].astype(f16), ew1b=ew1[D:].astype(f16),
        ew2=ew2.astype(f16),
        nw1a=nw1[:D].astype(f16), nw1b=nw1[D:].astype(f16),
        nw2=nw2.astype(f16),
        eb1=eb1.reshape(D, 1).astype(np.float32),
        eb2=eb2.reshape(D, 1).astype(np.float32),
        nb1=nb1.reshape(D, 1).astype(np.float32),
        nb2=nb2.reshape(D, 1).astype(np.float32),
    )
    in_maps = []
    for c in range(NCORES):
        sc = per_core_sched[c]
        xlocp = np.zeros((LOC_ROWS, D), f16)
        xlocp[:NPC] = x16[c * NPC:(c + 1) * NPC]
        m = dict(shared)
        m.update(
            xlocp=xlocp,
            xloc32=xpad[c * NPC:(c + 1) * NPC].copy(),
            cidxA=sc["colA"], cidxB=sc["colB"],
            rowrel=sc["rowrel"].astype(f16), maskt=sc["maskt"],
        )
        in_maps.append(m)
    return in_maps


_CACHE = {}
LAST_RESULT = None


def kernel(x, edge_index, edge_mask, ew1, eb1, ew2, eb2, nw1, nb1, nw2, nb2):
    x = np.asarray(x, np.float32)
    edge_index = np.asarray(edge_index)
    edge_mask = np.asarray(edge_mask, np.float32)

    T_A, T_B, sched = _preprocess(edge_index, edge_mask)
    in_maps = _make_inputs(np.asarray(x, np.float32), sched,
                           np.asarray(ew1, np.float32), np.asarray(eb1, np.float32),
                           np.asarray(ew2, np.float32), np.asarray(eb2, np.float32),
                           np.asarray(nw1, np.float32), np.asarray(nb1, np.float32),
                           np.asarray(nw2, np.float32), np.asarray(nb2, np.float32))

    key = (T_A, T_B)
    if key not in _CACHE:
        _CACHE[key] = _build_bass(T_A, T_B)
    nc = _CACHE[key]

    from concourse.bass_utils import run_bass_kernel_spmd
    global LAST_RESULT
    LAST_RESULT = run_bass_kernel_spmd(nc, in_maps, list(range(NCORES)))
    res = LAST_RESULT.results

    out = np.zeros((N, D), np.float32)
    edge_feat = np.zeros((E, D), np.float32)
    for c in range(NCORES):
        n0 = c * NPC
        n1 = min((c + 1) * NPC, N)
        out[n0:n1] = res[c]["outp"][:n1 - n0]
        orig = sched[c]["orig"]
        sel = orig >= 0
        edge_feat[orig[sel]] = res[c]["efeat"][sel].astype(np.float32)
    return out, edge_feat
